# revision 50
# baseline (speedup 1.0000x reference)
"""Trainium2 Bass kernel for the CapsuleLayer routing problem.

Strategy (i-sharded, 3 collectives — see kernel_v1_baseline.py for history):
  - Shard in_nodes (i) across the 8 cores: each core owns I_LOC = 144 input
    capsules, holding x[:, shard, :] (as both [b,(i,k)] and its transpose)
    and W[shard] packed as W_big[(i,k), (j,d)].
  - Per routing iteration, s[b,(j,d)] = x_flat @ (c ⊙ W_big) is a dense
    1152-deep matmul per core (partial over i), summed with ONE AllReduce.
    Every core squashes redundantly, computes P = x_flat^T @ v_flat, and
    updates its local b-logits from W_big ⊙ P (Hadamard + segmented reduce
    + a block-ones matmul for the k-sum / broadcast / 1/B scale).
  - 3rd iteration: ReduceScatter instead; each core squashes its 32-batch
    slice and writes it out; the host concatenates.

Optimizations vs the 145.9us baseline (now ~122us steady-state; end-to-end
rel err ~6e-3 vs the 2e-2 gate, validated in numpy + MultiCoreSim + HW):
  - Everything 16-bit: matmuls in bf16 streaming only the JD=160 moving
    columns (MM cost is N-columns cycles; the baseline's f32r needed a
    256-col pad for its fast path). PSUM tiles keep a 256-col stride so no
    matmul output straddles a 2KB PSUM bank. All three collectives carry
    bf16 (half wire bytes + half staging-DMA time).
  - Single ACT table set: natural_log_exp_and_others holds Ln AND Exp, so
    get_activation_tables is patched to resolve Exp/Ln only to that set —
    the baseline paid ~1.28us ACT_TABLE_LOAD thrash per squash/softmax.
  - Squash via ACT free affine: f = exp(.5*ln(a*sq) + bias)/(1 + a*sq)
    with the iter-0 uniform-c scaling folded into scale/bias for free;
    Square/den on the otherwise-idle scalar engine.
  - AllReduce buffers are partition-major [128, 2*JD] so the post-
    collective DMA-in is two contiguous-row transfers (the batch-major
    [B, JD] layout forced a slow strided gather); the ReduceScatter keeps
    batch-major rows so core c receives batches [32c, 32c+32).
  - Per-3-tile-group pipelining: P-matmul groups use separate PSUM tiles
    so W (x) P / d-reduce run while later P groups are still on the PE;
    k-sum matmul + b-update + softmax + Wc Hadamard also per group so the
    next iteration's s-matmuls start as soon as group 0's c is ready.
  - Half-0 staging cast on the scalar engine while half-1 matmuls run;
    input loads chunked 1/2/3-tile across three DMA queues so the first
    s-matmul starts ~3us earlier.
Known floor: ~22-28us of cross-core execution-launch skew (absorbed by the
framework's entry barrier) + ~11us one-time collective setup + ~11-15us
per RDH collective are runtime costs this kernel cannot remove; HAM keeps
the PE at 1.2GHz because every matmul burst is shorter than the 3.4us
warm-up window.
"""
import sys

for _p in ("/opt/trn_rl_repo",):
    if _p not in sys.path:
        sys.path.insert(0, _p)

import math

import numpy as np
import ml_dtypes

import concourse.bass as bass
import concourse.bacc as bacc
import concourse.mybir as mybir
import concourse.tile as tile
import concourse.hw_specs as hw_specs
from concourse.bass_utils import run_bass_kernel_spmd

F32 = mybir.dt.float32
BF16 = mybir.dt.bfloat16
AF = mybir.ActivationFunctionType
ALU = mybir.AluOpType

IN_NODES, OUT_NODES = 1152, 10
IN_DIM, OUT_DIM = 8, 16
B = 256
N_CORES = 8
ITERS = 3
I_LOC = IN_NODES // N_CORES          # 144
IK = I_LOC * IN_DIM                  # 1152
NT = IK // 128                       # 9 sbuf tiles over the (i,k) axis
JD = OUT_NODES * OUT_DIM             # 160
B_LOC = B // N_CORES                 # 32
RG = [list(range(N_CORES))]

# --- one ACT table set for the whole kernel ------------------------------
# natural_log_exp_and_others contains both Ln and Exp; by stripping those
# two functions from every other set's membership, the insert_act_table_
# loads pass can only ever pick that set, so the table is loaded once at
# startup instead of thrashing Ln<->Exp sets (~1.28us per reload).
_orig_get_tables = hw_specs.get_activation_tables


def _single_set_tables(arch):
    out = {}
    for name, fns in _orig_get_tables(arch).items():
        fns = set(fns)
        if name != "natural_log_exp_and_others":
            fns.discard(AF.Exp)
            fns.discard(AF.Ln)
        out[name] = fns
    return out


hw_specs.get_activation_tables = _single_set_tables
bacc.get_activation_tables = _single_set_tables  # bacc binds the name directly


def build_nc():
    nc = bacc.Bacc(
        "TRN2",
        target_bir_lowering=False,
        debug=False,
        enable_asserts=False,
        num_devices=N_CORES,
    )
    xT_d = nc.dram_tensor("xT", [2, NT, 128, 128], BF16,
                          kind="ExternalInput")
    xb_d = nc.dram_tensor("xb", [2, 128, IK], BF16, kind="ExternalInput")
    wb_d = nc.dram_tensor("wb", [NT, 128, JD], BF16, kind="ExternalInput")
    ones_d = nc.dram_tensor("onesb", [128, 128], BF16, kind="ExternalInput")
    out_d = nc.dram_tensor("out", [B_LOC, JD], F32, kind="ExternalOutput")

    with tile.TileContext(nc) as tc:
        with (
            tc.tile_pool(name="big", bufs=1) as bigp,
            tc.tile_pool(name="work", bufs=2) as workp,
            tc.tile_pool(name="psum", bufs=2, space="PSUM") as psum,
            tc.tile_pool(name="dram", bufs=2, space="DRAM") as dramp,
        ):
            W_sb = bigp.tile([128, NT, JD], BF16)
            Wc_sb = bigp.tile([128, NT, JD], BF16)
            xT_sb = bigp.tile([128, NT * B], BF16)        # (128, 2304)
            x_sb = bigp.tile([128, 2 * IK], BF16)         # (128, 2304)
            ones_sb = bigp.tile([128, 128], BF16)
            b_sb = bigp.tile([128, NT * OUT_NODES], F32)  # (128, 90) logits

            # chunked strided loads, spread across engine queues in
            # matmul-need order. Iteration 0 runs batch-half 0 first, so
            # only W and xT[half 0] gate the first 9 matmuls; xT[half 1]
            # streams while half 0 computes (the AR#1 doorbell is on the
            # critical path: last-rank doorbell + ~10us RDH = AR1 end).
            xT_v = (xT_sb[:].rearrange("p (t g c) -> p t g c", g=2, c=128))

            def _w_chunk(eng, t0, t1):
                eng.dma_start(W_sb[:, t0:t1, :],
                              wb_d[t0:t1].rearrange("t p x -> p t x"))

            def _x_chunk(eng, b0, t0, t1):
                eng.dma_start(
                    xT_v[:, t0:t1, b0, :],
                    xT_d[b0, t0:t1].rearrange("t p c -> p t c"))

            _w_chunk(nc.sync, 0, 1)
            _x_chunk(nc.scalar, 0, 0, 1)
            _x_chunk(nc.gpsimd, 0, 1, 5)
            _w_chunk(nc.scalar, 1, 3)
            _w_chunk(nc.sync, 3, 6)
            _x_chunk(nc.scalar, 0, 5, 9)
            _w_chunk(nc.gpsimd, 6, 9)
            _x_chunk(nc.sync, 1, 0, 5)
            _x_chunk(nc.scalar, 1, 5, 9)
            nc.gpsimd.dma_start(ones_sb[:], ones_d[:])
            h_xb = nc.gpsimd.dma_start(
                x_sb[:].rearrange("p (g i) -> p g i", i=IK),
                xb_d[:].rearrange("g p i -> p g i"))
            nc.gpsimd.memset(b_sb[:], 0.0)
            # per-partition scalar bias ln(0.1) for the iter-0 squash
            lnb = bigp.tile([128, 1], F32)
            nc.gpsimd.memset(lnb[:], math.log(0.1))
            # prime the (single) ACT table EARLY, from a memset tile (not
            # ones_sb, which now loads last) — otherwise the table load
            # slides into the iteration-0 staging path
            tprime = workp.tile([128, 8], F32, tag="tprime")
            nc.scalar.activation(tprime[:], b_sb[:, 0:8], AF.Exp)

            for it in range(ITERS):
                rhs_sb = W_sb if it == 0 else Wc_sb
                # ---- s-matmul: s[b, (j,d)] partial over local i ----
                s_ps = psum.tile([128, 2, 256], F32, tag="s_ps", bufs=1)
                is_last = it == ITERS - 1
                # mid iterations AllReduce bf16 in a partition-major
                # [128, 2*JD] layout (contiguous 640B rows -> the post-
                # collective DMA-in is one fast transfer); the final
                # ReduceScatter needs batch-major [B, JD] rows in fp32 so
                # each core receives its own 32-batch slice.
                if is_last:
                    sin = dramp.tile([B, JD], BF16, tag="cc_in_rs")
                else:
                    sin = dramp.tile([128, 2 * JD], BF16, tag="cc_in")
                h_sin1 = None
                for b0 in range(2):
                    for t in range(NT):
                        nc.tensor.matmul(
                            s_ps[:, b0, 0:JD],
                            xT_sb[:, t * B + b0 * 128:
                                  t * B + b0 * 128 + 128],
                            rhs_sb[:, t, :],
                            start=(t == 0),
                            stop=(t == NT - 1),
                        )
                    # stage this half while the other half's matmuls run;
                    # half 0's cast goes on the idle scalar engine — except
                    # in iteration 0, where the scalar queue may carry the
                    # one-time ACT table load: keep iter-0 staging off it
                    s_st = workp.tile([128, JD], BF16, tag=f"s_st{b0}")
                    if b0 == 0 and it > 0:
                        nc.scalar.activation(s_st[:], s_ps[:, b0, 0:JD],
                                             AF.Copy)
                    else:
                        nc.vector.tensor_copy(s_st[:], s_ps[:, b0, 0:JD])
                    if b0 == 0:
                        eng = nc.sync
                    else:
                        eng = nc.gpsimd if it == 0 else nc.scalar
                    dst = (sin[b0 * 128:(b0 + 1) * 128, :] if is_last
                           else sin[:, b0 * JD:(b0 + 1) * JD])
                    h_sin1 = eng.dma_start(dst, s_st[:])
                if it == 0:
                    # keep the 1.2MB x load off the critical DMA path
                    bass._add_dep_helper(
                        h_xb.ins, h_sin1.ins, sync=True,
                        reason="defer x load until s staged")
                if not is_last:
                    sout = dramp.tile([128, 2 * JD], BF16, tag="cc_out",
                                      addr_space="Shared")
                    nc.gpsimd.collective_compute(
                        "AllReduce", ALU.add, replica_groups=RG,
                        ins=[sin[:]], outs=[sout[:]],
                    )
                    # ---- contiguous-row DMA in (2 queues), batched squash
                    s_h = workp.tile([128, 2, JD], BF16, tag="s_h")
                    sout_v = sout[:].rearrange("p (g j) -> p g j", j=JD)
                    nc.sync.dma_start(s_h[0:64], sout_v[0:64])
                    nc.scalar.dma_start(s_h[64:128], sout_v[64:128])
                    # iteration 0 runs on raw W (c uniform 1/10): fold the
                    # scaling into the squash via ACT's free affine:
                    # f = exp(.5*ln(a*sq) + ln(.1)) / (1 + a*sq), a = 0.01.
                    a = 0.01 if it == 0 else 1.0
                    ssq = workp.tile([128, 2, JD], F32, tag="ssq")
                    nc.scalar.activation(ssq[:], s_h[:], AF.Square)
                    sq = workp.tile([128, 2 * OUT_NODES], F32, tag="sq")
                    nc.vector.tensor_reduce(
                        sq[:],
                        ssq[:].rearrange("p g (j d) -> p g j d", d=OUT_DIM),
                        axis=mybir.AxisListType.X, op=ALU.add,
                    )
                    rt = workp.tile([128, 2 * OUT_NODES], F32, tag="rt")
                    nc.scalar.activation(rt[:], sq[:], AF.Ln, scale=a)
                    rt2 = workp.tile([128, 2 * OUT_NODES], F32, tag="rt2")
                    nc.scalar.activation(
                        rt2[:], rt[:], AF.Exp, scale=0.5,
                        bias=(lnb[:] if it == 0 else 0.0))
                    den = workp.tile([128, 2 * OUT_NODES], F32, tag="den")
                    # den inline on vector: it feeds recip on the serial
                    # squash path, and the scalar detour costs a sem hop
                    nc.vector.tensor_scalar(den[:], sq[:], a, 1.0,
                                            op0=ALU.mult, op1=ALU.add)
                    dri = workp.tile([128, 2 * OUT_NODES], F32, tag="dri")
                    nc.vector.reciprocal(dri[:], den[:])
                    f = workp.tile([128, 2 * OUT_NODES], F32, tag="f")
                    nc.vector.tensor_tensor(f[:], rt2[:], dri[:],
                                            op=ALU.mult)
                    # v written per batch-half in two separate tiles so the
                    # P-matmuls' b0=0 pass starts as soon as half 0 is done
                    # instead of waiting on the whole-tensor write
                    v16 = [workp.tile([128, JD], BF16, tag=f"v16_{g}",
                                      name=f"v16_{g}")
                           for g in range(2)]
                    f_v = f[:].rearrange("p (g j) -> p g j", j=OUT_NODES)
                    for g in range(2):
                        f_b = (f_v[:, g, :].unsqueeze(2)
                               .broadcast_to([128, OUT_NODES, OUT_DIM]))
                        nc.vector.tensor_tensor(
                            v16[g][:].rearrange("p (j d) -> p j d",
                                                d=OUT_DIM),
                            s_h[:, g, :].rearrange("p (j d) -> p j d",
                                                   d=OUT_DIM),
                            f_b, op=ALU.mult,
                        )
                    # ---- P = x^T @ v ; Y = reduce_d(W ⊙ P) ; k-sum ----
                    # one PSUM tile per 3-tile group so the z/y vector ops
                    # pipeline with the remaining P matmuls
                    y_all = workp.tile([128, NT * OUT_NODES], BF16,
                                       tag="y_all")
                    for g3 in range(3):
                        t0 = 3 * g3
                        pp_g = psum.tile([128, 3, 256], F32,
                                         tag=f"pp{g3}", bufs=1)
                        for dt3 in range(3):
                            t = t0 + dt3
                            for b0 in range(2):
                                nc.tensor.matmul(
                                    pp_g[:, dt3, 0:JD],
                                    x_sb[:, b0 * IK + t * 128:
                                         b0 * IK + t * 128 + 128],
                                    v16[b0][:],
                                    start=(b0 == 0),
                                    stop=(b0 == 1),
                                )
                        z_g = workp.tile([128, 3, JD], BF16, tag=f"z{g3}")
                        with nc.allow_low_precision(
                                reason="bf16 z/y feed routing logits "
                                "only; validated rel err ~5e-3 vs 2e-2"):
                            nc.vector.tensor_tensor(
                                z_g[:], W_sb[:, t0:t0 + 3, :],
                                pp_g[:, :, 0:JD], op=ALU.mult,
                            )
                            # 3D contiguous view keeps the packed 2x mode
                            nc.vector.tensor_reduce(
                                y_all[:, t0 * OUT_NODES:
                                      (t0 + 3) * OUT_NODES],
                                z_g[:].rearrange("p t (q d) -> p (t q) d",
                                                 d=OUT_DIM),
                                axis=mybir.AxisListType.X, op=ALU.add,
                            )
                    # ---- per 3-tile group: k-sum matmul (ones_sb holds
                    # 1/B in 8x8 diag blocks), b += upd, softmax, Wc —
                    # so the next iteration's s-matmuls on group g start
                    # as soon as group g's b-path is done, not after the
                    # whole softmax.
                    y_ps = psum.tile([128, NT * OUT_NODES], F32,
                                     tag="y_ps", bufs=1)
                    e = workp.tile([128, NT * OUT_NODES], F32, tag="e")
                    for g3 in range(3):
                        j0 = 3 * g3 * OUT_NODES
                        j1 = j0 + 3 * OUT_NODES
                        t0 = 3 * g3
                        nc.tensor.matmul(y_ps[:, j0:j1], ones_sb[:],
                                         y_all[:, j0:j1],
                                         start=True, stop=True)
                        nc.vector.tensor_tensor(b_sb[:, j0:j1],
                                                b_sb[:, j0:j1],
                                                y_ps[:, j0:j1], op=ALU.add)
                        nc.scalar.activation(e[:, j0:j1], b_sb[:, j0:j1],
                                             AF.Exp)
                        dsum = workp.tile([128, 3], F32, tag=f"dsum{g3}")
                        nc.vector.tensor_reduce(
                            dsum[:],
                            e[:, j0:j1].rearrange("p (t j) -> p t j",
                                                  j=OUT_NODES),
                            axis=mybir.AxisListType.X, op=ALU.add,
                        )
                        r = workp.tile([128, 3], F32, tag=f"r{g3}")
                        nc.vector.reciprocal(r[:], dsum[:])
                        c = workp.tile([128, 3 * OUT_NODES], F32,
                                       tag=f"c{g3}")
                        r_b = r[:].unsqueeze(2).broadcast_to(
                            [128, 3, OUT_NODES])
                        nc.vector.tensor_tensor(
                            c[:].rearrange("p (t j) -> p t j",
                                           j=OUT_NODES),
                            e[:, j0:j1].rearrange("p (t j) -> p t j",
                                                  j=OUT_NODES),
                            r_b, op=ALU.mult,
                        )
                        c_b = (c[:].rearrange("p (t j) -> p t j",
                                              j=OUT_NODES)
                               .unsqueeze(3)
                               .broadcast_to([128, 3, OUT_NODES, OUT_DIM]))
                        nc.vector.tensor_tensor(
                            Wc_sb[:, t0:t0 + 3, :].rearrange(
                                "p t (j d) -> p t j d", d=OUT_DIM),
                            W_sb[:, t0:t0 + 3, :].rearrange(
                                "p t (j d) -> p t j d", d=OUT_DIM),
                            c_b, op=ALU.mult,
                        )
                else:
                    # ---- final iter: ReduceScatter (bf16), squash own
                    # 32-batch slice, write out ----
                    sout_rs = dramp.tile([B_LOC, JD], BF16,
                                         tag="cc_out_rs")
                    nc.gpsimd.collective_compute(
                        "ReduceScatter", ALU.add, replica_groups=RG,
                        ins=[sin[:]], outs=[sout_rs[:]],
                    )
                    sl = workp.tile([B_LOC, JD], BF16, tag="sl")
                    nc.sync.dma_start(sl[:], sout_rs[:])
                    ssq_l = workp.tile([B_LOC, JD], F32, tag="ssq_l")
                    nc.scalar.activation(ssq_l[:], sl[:], AF.Square)
                    sq_l = workp.tile([B_LOC, OUT_NODES], F32, tag="sq_l")
                    nc.vector.tensor_reduce(
                        sq_l[:],
                        ssq_l[:].rearrange("p (j d) -> p j d", d=OUT_DIM),
                        axis=mybir.AxisListType.X, op=ALU.add,
                    )
                    rt_l = workp.tile([B_LOC, OUT_NODES], F32, tag="rt_l")
                    nc.scalar.activation(rt_l[:], sq_l[:], AF.Ln)
                    rt2_l = workp.tile([B_LOC, OUT_NODES], F32,
                                       tag="rt2_l")
                    nc.scalar.activation(rt2_l[:], rt_l[:], AF.Exp,
                                         scale=0.5)
                    den_l = workp.tile([B_LOC, OUT_NODES], F32,
                                       tag="den_l")
                    nc.vector.tensor_scalar_add(den_l[:], sq_l[:], 1.0)
                    dri_l = workp.tile([B_LOC, OUT_NODES], F32,
                                       tag="dri_l")
                    nc.vector.reciprocal(dri_l[:], den_l[:])
                    f_l = workp.tile([B_LOC, OUT_NODES], F32, tag="f_l")
                    nc.vector.tensor_tensor(f_l[:], rt2_l[:], dri_l[:],
                                            op=ALU.mult)
                    v_l = workp.tile([B_LOC, JD], F32, tag="v_l")
                    f_lb = (f_l[:].unsqueeze(2)
                            .broadcast_to([B_LOC, OUT_NODES, OUT_DIM]))
                    nc.vector.tensor_tensor(
                        v_l[:].rearrange("p (j d) -> p j d", d=OUT_DIM),
                        sl[:].rearrange("p (j d) -> p j d", d=OUT_DIM),
                        f_lb, op=ALU.mult,
                    )
                    nc.sync.dma_start(out_d[:], v_l[:])

    nc.compile()
    return nc


def make_inmaps(x, W):
    x = np.ascontiguousarray(np.asarray(x, dtype=np.float32))
    W = np.ascontiguousarray(np.asarray(W, dtype=np.float32))
    # 16 8x8 blocks of 1/B on the diagonal
    ones_blk = (np.kron(np.eye(128 // IN_DIM, dtype=np.float32),
                        np.ones((IN_DIM, IN_DIM), dtype=np.float32)) / B)
    in_maps = []
    for cid in range(N_CORES):
        sh = slice(cid * I_LOC, (cid + 1) * I_LOC)
        x_sh = x[:, sh, :].reshape(B, IK)
        # [2, NT, 128, 128]: batch-half-major so iteration 0's half-0
        # matmuls are gated by only half the xT bytes
        xT = np.ascontiguousarray(
            x_sh.T.astype(ml_dtypes.bfloat16)
            .reshape(NT, 128, 2, 128).transpose(2, 0, 1, 3))
        xb = x_sh.astype(ml_dtypes.bfloat16).reshape(2, 128, IK)
        wb = W[sh].transpose(0, 3, 1, 2).reshape(NT, 128, JD).astype(
            ml_dtypes.bfloat16)
        in_maps.append({
            "xT": xT, "xb": xb, "wb": wb,
            "onesb": ones_blk.astype(ml_dtypes.bfloat16),
        })
    return in_maps


def assemble_output(per_core_outs):
    slices = [per_core_outs[c]["out"].reshape(B_LOC, OUT_NODES, OUT_DIM)
              for c in range(N_CORES)]
    v = np.concatenate(slices, axis=0)        # (256, 10, 16), already [b, j, d]
    return v[..., None].astype(np.float32)    # (256, 10, 16, 1)


_CACHED_NC = None


def kernel(x=None, W=None, **kw):
    global _CACHED_NC
    if x is None:
        x = kw["x"]
    if W is None:
        W = kw["W"]
    if _CACHED_NC is None:
        _CACHED_NC = build_nc()
    in_maps = make_inmaps(x, W)
    res = run_bass_kernel_spmd(
        _CACHED_NC, in_maps, core_ids=list(range(N_CORES)))
    return assemble_output(res.results)


if __name__ == "__main__":
    nc = build_nc()
    print("build + compile OK")


# revision 51
# speedup vs baseline: 1.0129x; 1.0129x over previous
"""Trainium2 Bass kernel for the CapsuleLayer routing problem.

Strategy (i-sharded, 3 collectives — see kernel_v1_baseline.py for history):
  - Shard in_nodes (i) across the 8 cores: each core owns I_LOC = 144 input
    capsules, holding x[:, shard, :] (as both [b,(i,k)] and its transpose)
    and W[shard] packed as W_big[(i,k), (j,d)].
  - Per routing iteration, s[b,(j,d)] = x_flat @ (c ⊙ W_big) is a dense
    1152-deep matmul per core (partial over i), summed with ONE AllReduce.
    Every core squashes redundantly, computes P = x_flat^T @ v_flat, and
    updates its local b-logits from W_big ⊙ P (Hadamard + segmented reduce
    + a block-ones matmul for the k-sum / broadcast / 1/B scale).
  - 3rd iteration: ReduceScatter instead; each core squashes its 32-batch
    slice and writes it out; the host concatenates.

Optimizations vs the 145.9us baseline (now ~122us steady-state; end-to-end
rel err ~6e-3 vs the 2e-2 gate, validated in numpy + MultiCoreSim + HW):
  - Everything 16-bit: matmuls in bf16 streaming only the JD=160 moving
    columns (MM cost is N-columns cycles; the baseline's f32r needed a
    256-col pad for its fast path). PSUM tiles keep a 256-col stride so no
    matmul output straddles a 2KB PSUM bank. All three collectives carry
    bf16 (half wire bytes + half staging-DMA time).
  - Single ACT table set: natural_log_exp_and_others holds Ln AND Exp, so
    get_activation_tables is patched to resolve Exp/Ln only to that set —
    the baseline paid ~1.28us ACT_TABLE_LOAD thrash per squash/softmax.
  - Squash via ACT free affine: f = exp(.5*ln(a*sq) + bias)/(1 + a*sq)
    with the iter-0 uniform-c scaling folded into scale/bias for free;
    Square/den on the otherwise-idle scalar engine.
  - AllReduce buffers are partition-major [128, 2*JD] so the post-
    collective DMA-in is two contiguous-row transfers (the batch-major
    [B, JD] layout forced a slow strided gather); the ReduceScatter keeps
    batch-major rows so core c receives batches [32c, 32c+32).
  - Per-3-tile-group pipelining: P-matmul groups use separate PSUM tiles
    so W (x) P / d-reduce run while later P groups are still on the PE;
    k-sum matmul + b-update + softmax + Wc Hadamard also per group so the
    next iteration's s-matmuls start as soon as group 0's c is ready.
  - Half-0 staging cast on the scalar engine while half-1 matmuls run;
    input loads chunked 1/2/3-tile across three DMA queues so the first
    s-matmul starts ~3us earlier.
Known floor: ~22-28us of cross-core execution-launch skew (absorbed by the
framework's entry barrier) + ~11us one-time collective setup + ~11-15us
per RDH collective are runtime costs this kernel cannot remove; HAM keeps
the PE at 1.2GHz because every matmul burst is shorter than the 3.4us
warm-up window.
"""
import sys

for _p in ("/opt/trn_rl_repo",):
    if _p not in sys.path:
        sys.path.insert(0, _p)

import math

import numpy as np
import ml_dtypes

import concourse.bass as bass
import concourse.bacc as bacc
import concourse.mybir as mybir
import concourse.tile as tile
import concourse.hw_specs as hw_specs
from concourse.bass_utils import run_bass_kernel_spmd

F32 = mybir.dt.float32
BF16 = mybir.dt.bfloat16
AF = mybir.ActivationFunctionType
ALU = mybir.AluOpType

IN_NODES, OUT_NODES = 1152, 10
IN_DIM, OUT_DIM = 8, 16
B = 256
N_CORES = 8
ITERS = 3
I_LOC = IN_NODES // N_CORES          # 144
IK = I_LOC * IN_DIM                  # 1152
NT = IK // 128                       # 9 sbuf tiles over the (i,k) axis
JD = OUT_NODES * OUT_DIM             # 160
B_LOC = B // N_CORES                 # 32
RG = [list(range(N_CORES))]

# --- one ACT table set for the whole kernel ------------------------------
# natural_log_exp_and_others contains both Ln and Exp; by stripping those
# two functions from every other set's membership, the insert_act_table_
# loads pass can only ever pick that set, so the table is loaded once at
# startup instead of thrashing Ln<->Exp sets (~1.28us per reload).
_orig_get_tables = hw_specs.get_activation_tables


def _single_set_tables(arch):
    out = {}
    for name, fns in _orig_get_tables(arch).items():
        fns = set(fns)
        if name != "natural_log_exp_and_others":
            fns.discard(AF.Exp)
            fns.discard(AF.Ln)
        out[name] = fns
    return out


hw_specs.get_activation_tables = _single_set_tables
bacc.get_activation_tables = _single_set_tables  # bacc binds the name directly


def build_nc():
    nc = bacc.Bacc(
        "TRN2",
        target_bir_lowering=False,
        debug=False,
        enable_asserts=False,
        num_devices=N_CORES,
    )
    xT_d = nc.dram_tensor("xT", [2, NT, 128, 128], BF16,
                          kind="ExternalInput")
    xb_d = nc.dram_tensor("xb", [2, 128, IK], BF16, kind="ExternalInput")
    wb_d = nc.dram_tensor("wb", [NT, 128, JD], BF16, kind="ExternalInput")
    ones_d = nc.dram_tensor("onesb", [128, 128], BF16, kind="ExternalInput")
    out_d = nc.dram_tensor("out", [B_LOC, JD], F32, kind="ExternalOutput")

    with tile.TileContext(nc) as tc:
        with (
            tc.tile_pool(name="big", bufs=1) as bigp,
            tc.tile_pool(name="work", bufs=2) as workp,
            tc.tile_pool(name="psum", bufs=2, space="PSUM") as psum,
            tc.tile_pool(name="dram", bufs=2, space="DRAM") as dramp,
        ):
            W_sb = bigp.tile([128, NT, JD], BF16)
            Wc_sb = bigp.tile([128, NT, JD], BF16)
            xT_sb = bigp.tile([128, NT * B], BF16)        # (128, 2304)
            x_sb = bigp.tile([128, 2 * IK], BF16)         # (128, 2304)
            ones_sb = bigp.tile([128, 128], BF16)
            b_sb = bigp.tile([128, NT * OUT_NODES], F32)  # (128, 90) logits

            # chunked strided loads, spread across engine queues in
            # matmul-need order. Iteration 0 runs batch-half 0 first, so
            # only W and xT[half 0] gate the first 9 matmuls; xT[half 1]
            # streams while half 0 computes (the AR#1 doorbell is on the
            # critical path: last-rank doorbell + ~10us RDH = AR1 end).
            xT_v = (xT_sb[:].rearrange("p (t g c) -> p t g c", g=2, c=128))

            def _w_chunk(eng, t0, t1):
                eng.dma_start(W_sb[:, t0:t1, :],
                              wb_d[t0:t1].rearrange("t p x -> p t x"))

            def _x_chunk(eng, b0, t0, t1):
                eng.dma_start(
                    xT_v[:, t0:t1, b0, :],
                    xT_d[b0, t0:t1].rearrange("t p c -> p t c"))

            _w_chunk(nc.sync, 0, 1)
            _x_chunk(nc.scalar, 0, 0, 1)
            _x_chunk(nc.gpsimd, 0, 1, 5)
            _w_chunk(nc.scalar, 1, 3)
            _w_chunk(nc.sync, 3, 6)
            _x_chunk(nc.scalar, 0, 5, 9)
            _w_chunk(nc.gpsimd, 6, 9)
            _x_chunk(nc.sync, 1, 0, 5)
            _x_chunk(nc.scalar, 1, 5, 9)
            nc.gpsimd.dma_start(ones_sb[:], ones_d[:])
            h_xb = nc.gpsimd.dma_start(
                x_sb[:].rearrange("p (g i) -> p g i", i=IK),
                xb_d[:].rearrange("g p i -> p g i"))
            nc.gpsimd.memset(b_sb[:], 0.0)
            # per-partition scalar bias ln(0.1) for the iter-0 squash
            lnb = bigp.tile([128, 1], F32)
            nc.gpsimd.memset(lnb[:], math.log(0.1))
            # prime the (single) ACT table EARLY, from a memset tile (not
            # ones_sb, which now loads last) — otherwise the table load
            # slides into the iteration-0 staging path
            tprime = workp.tile([128, 8], F32, tag="tprime")
            nc.scalar.activation(tprime[:], b_sb[:, 0:8], AF.Exp)

            for it in range(ITERS):
                rhs_sb = W_sb if it == 0 else Wc_sb
                # ---- s-matmul: s[b, (j,d)] partial over local i ----
                s_ps = psum.tile([128, 2, 256], F32, tag="s_ps", bufs=1)
                is_last = it == ITERS - 1
                # mid iterations AllReduce bf16 in a partition-major
                # [128, 2*JD] layout (contiguous 640B rows -> the post-
                # collective DMA-in is one fast transfer); the final
                # ReduceScatter needs batch-major [B, JD] rows in fp32 so
                # each core receives its own 32-batch slice.
                if is_last:
                    sin = dramp.tile([B, JD], BF16, tag="cc_in_rs")
                else:
                    sin = dramp.tile([128, 2 * JD], BF16, tag="cc_in")
                h_sin1 = None
                for b0 in range(2):
                    for t in range(NT):
                        nc.tensor.matmul(
                            s_ps[:, b0, 0:JD],
                            xT_sb[:, t * B + b0 * 128:
                                  t * B + b0 * 128 + 128],
                            rhs_sb[:, t, :],
                            start=(t == 0),
                            stop=(t == NT - 1),
                        )
                    # stage this half while the other half's matmuls run;
                    # half 0's cast goes on the idle scalar engine — except
                    # in iteration 0, where the scalar queue may carry the
                    # one-time ACT table load: keep iter-0 staging off it
                    s_st = workp.tile([128, JD], BF16, tag=f"s_st{b0}")
                    if b0 == 0 and it > 0:
                        nc.scalar.activation(s_st[:], s_ps[:, b0, 0:JD],
                                             AF.Copy)
                    else:
                        nc.vector.tensor_copy(s_st[:], s_ps[:, b0, 0:JD])
                    if b0 == 0:
                        eng = nc.sync
                    else:
                        eng = nc.gpsimd if it == 0 else nc.scalar
                    dst = (sin[b0 * 128:(b0 + 1) * 128, :] if is_last
                           else sin[:, b0 * JD:(b0 + 1) * JD])
                    h_sin1 = eng.dma_start(dst, s_st[:])
                if it == 0:
                    # keep the 1.2MB x load off the critical DMA path
                    bass._add_dep_helper(
                        h_xb.ins, h_sin1.ins, sync=True,
                        reason="defer x load until s staged")
                if not is_last:
                    sout = dramp.tile([128, 2 * JD], BF16, tag="cc_out",
                                      addr_space="Shared")
                    nc.gpsimd.collective_compute(
                        "AllReduce", ALU.add, replica_groups=RG,
                        ins=[sin[:]], outs=[sout[:]],
                    )
                    # ---- contiguous-row DMA in (2 queues), batched squash
                    s_h = workp.tile([128, 2, JD], BF16, tag="s_h")
                    sout_v = sout[:].rearrange("p (g j) -> p g j", j=JD)
                    nc.sync.dma_start(s_h[0:64], sout_v[0:64])
                    nc.scalar.dma_start(s_h[64:128], sout_v[64:128])
                    # iteration 0 runs on raw W (c uniform 1/10): fold the
                    # scaling into the squash via ACT's free affine:
                    # f = exp(.5*ln(a*sq) + ln(.1)) / (1 + a*sq), a = 0.01.
                    a = 0.01 if it == 0 else 1.0
                    ssq = workp.tile([128, 2, JD], F32, tag="ssq")
                    nc.scalar.activation(ssq[:], s_h[:], AF.Square)
                    sq = workp.tile([128, 2 * OUT_NODES], F32, tag="sq")
                    nc.vector.tensor_reduce(
                        sq[:],
                        ssq[:].rearrange("p g (j d) -> p g j d", d=OUT_DIM),
                        axis=mybir.AxisListType.X, op=ALU.add,
                    )
                    rt = workp.tile([128, 2 * OUT_NODES], F32, tag="rt")
                    nc.scalar.activation(rt[:], sq[:], AF.Ln, scale=a)
                    rt2 = workp.tile([128, 2 * OUT_NODES], F32, tag="rt2")
                    nc.scalar.activation(
                        rt2[:], rt[:], AF.Exp, scale=0.5,
                        bias=(lnb[:] if it == 0 else 0.0))
                    den = workp.tile([128, 2 * OUT_NODES], F32, tag="den")
                    # den inline on vector: it feeds recip on the serial
                    # squash path, and the scalar detour costs a sem hop
                    nc.vector.tensor_scalar(den[:], sq[:], a, 1.0,
                                            op0=ALU.mult, op1=ALU.add)
                    dri = workp.tile([128, 2 * OUT_NODES], F32, tag="dri")
                    nc.vector.reciprocal(dri[:], den[:])
                    f = workp.tile([128, 2 * OUT_NODES], F32, tag="f")
                    nc.vector.tensor_tensor(f[:], rt2[:], dri[:],
                                            op=ALU.mult)
                    v16 = workp.tile([128, 2, JD], BF16, tag="v16")
                    f_b = (f[:].rearrange("p (g j) -> p g j", j=OUT_NODES)
                           .unsqueeze(3)
                           .broadcast_to([128, 2, OUT_NODES, OUT_DIM]))
                    nc.vector.tensor_tensor(
                        v16[:].rearrange("p g (j d) -> p g j d", d=OUT_DIM),
                        s_h[:].rearrange("p g (j d) -> p g j d", d=OUT_DIM),
                        f_b, op=ALU.mult,
                    )
                    # ---- P = x^T @ v ; Y = reduce_d(W ⊙ P) ; k-sum ----
                    # one PSUM tile per 3-tile group so the z/y vector ops
                    # pipeline with the remaining P matmuls
                    y_all = workp.tile([128, NT * OUT_NODES], BF16,
                                       tag="y_all")
                    for g3 in range(3):
                        t0 = 3 * g3
                        pp_g = psum.tile([128, 3, 256], F32,
                                         tag=f"pp{g3}", bufs=1)
                        for dt3 in range(3):
                            t = t0 + dt3
                            for b0 in range(2):
                                nc.tensor.matmul(
                                    pp_g[:, dt3, 0:JD],
                                    x_sb[:, b0 * IK + t * 128:
                                         b0 * IK + t * 128 + 128],
                                    v16[:, b0, :],
                                    start=(b0 == 0),
                                    stop=(b0 == 1),
                                )
                        z_g = workp.tile([128, 3, JD], BF16, tag=f"z{g3}")
                        with nc.allow_low_precision(
                                reason="bf16 z/y feed routing logits "
                                "only; validated rel err ~5e-3 vs 2e-2"):
                            nc.vector.tensor_tensor(
                                z_g[:], W_sb[:, t0:t0 + 3, :],
                                pp_g[:, :, 0:JD], op=ALU.mult,
                            )
                            # 3D contiguous view keeps the packed 2x mode
                            nc.vector.tensor_reduce(
                                y_all[:, t0 * OUT_NODES:
                                      (t0 + 3) * OUT_NODES],
                                z_g[:].rearrange("p t (q d) -> p (t q) d",
                                                 d=OUT_DIM),
                                axis=mybir.AxisListType.X, op=ALU.add,
                            )
                    # ---- per 3-tile group: k-sum matmul (ones_sb holds
                    # 1/B in 8x8 diag blocks), b += upd, softmax, Wc —
                    # so the next iteration's s-matmuls on group g start
                    # as soon as group g's b-path is done, not after the
                    # whole softmax.
                    y_ps = psum.tile([128, NT * OUT_NODES], F32,
                                     tag="y_ps", bufs=1)
                    e = workp.tile([128, NT * OUT_NODES], F32, tag="e")
                    for g3 in range(3):
                        j0 = 3 * g3 * OUT_NODES
                        j1 = j0 + 3 * OUT_NODES
                        t0 = 3 * g3
                        nc.tensor.matmul(y_ps[:, j0:j1], ones_sb[:],
                                         y_all[:, j0:j1],
                                         start=True, stop=True)
                        nc.vector.tensor_tensor(b_sb[:, j0:j1],
                                                b_sb[:, j0:j1],
                                                y_ps[:, j0:j1], op=ALU.add)
                        nc.scalar.activation(e[:, j0:j1], b_sb[:, j0:j1],
                                             AF.Exp)
                        dsum = workp.tile([128, 3], F32, tag=f"dsum{g3}")
                        nc.vector.tensor_reduce(
                            dsum[:],
                            e[:, j0:j1].rearrange("p (t j) -> p t j",
                                                  j=OUT_NODES),
                            axis=mybir.AxisListType.X, op=ALU.add,
                        )
                        r = workp.tile([128, 3], F32, tag=f"r{g3}")
                        nc.vector.reciprocal(r[:], dsum[:])
                        c = workp.tile([128, 3 * OUT_NODES], F32,
                                       tag=f"c{g3}")
                        r_b = r[:].unsqueeze(2).broadcast_to(
                            [128, 3, OUT_NODES])
                        nc.vector.tensor_tensor(
                            c[:].rearrange("p (t j) -> p t j",
                                           j=OUT_NODES),
                            e[:, j0:j1].rearrange("p (t j) -> p t j",
                                                  j=OUT_NODES),
                            r_b, op=ALU.mult,
                        )
                        c_b = (c[:].rearrange("p (t j) -> p t j",
                                              j=OUT_NODES)
                               .unsqueeze(3)
                               .broadcast_to([128, 3, OUT_NODES, OUT_DIM]))
                        nc.vector.tensor_tensor(
                            Wc_sb[:, t0:t0 + 3, :].rearrange(
                                "p t (j d) -> p t j d", d=OUT_DIM),
                            W_sb[:, t0:t0 + 3, :].rearrange(
                                "p t (j d) -> p t j d", d=OUT_DIM),
                            c_b, op=ALU.mult,
                        )
                else:
                    # ---- final iter: ReduceScatter (bf16), squash own
                    # 32-batch slice, write out ----
                    sout_rs = dramp.tile([B_LOC, JD], BF16,
                                         tag="cc_out_rs")
                    nc.gpsimd.collective_compute(
                        "ReduceScatter", ALU.add, replica_groups=RG,
                        ins=[sin[:]], outs=[sout_rs[:]],
                    )
                    sl = workp.tile([B_LOC, JD], BF16, tag="sl")
                    nc.sync.dma_start(sl[:], sout_rs[:])
                    ssq_l = workp.tile([B_LOC, JD], F32, tag="ssq_l")
                    nc.scalar.activation(ssq_l[:], sl[:], AF.Square)
                    sq_l = workp.tile([B_LOC, OUT_NODES], F32, tag="sq_l")
                    nc.vector.tensor_reduce(
                        sq_l[:],
                        ssq_l[:].rearrange("p (j d) -> p j d", d=OUT_DIM),
                        axis=mybir.AxisListType.X, op=ALU.add,
                    )
                    rt_l = workp.tile([B_LOC, OUT_NODES], F32, tag="rt_l")
                    nc.scalar.activation(rt_l[:], sq_l[:], AF.Ln)
                    rt2_l = workp.tile([B_LOC, OUT_NODES], F32,
                                       tag="rt2_l")
                    nc.scalar.activation(rt2_l[:], rt_l[:], AF.Exp,
                                         scale=0.5)
                    den_l = workp.tile([B_LOC, OUT_NODES], F32,
                                       tag="den_l")
                    nc.vector.tensor_scalar_add(den_l[:], sq_l[:], 1.0)
                    dri_l = workp.tile([B_LOC, OUT_NODES], F32,
                                       tag="dri_l")
                    nc.vector.reciprocal(dri_l[:], den_l[:])
                    f_l = workp.tile([B_LOC, OUT_NODES], F32, tag="f_l")
                    nc.vector.tensor_tensor(f_l[:], rt2_l[:], dri_l[:],
                                            op=ALU.mult)
                    v_l = workp.tile([B_LOC, JD], F32, tag="v_l")
                    f_lb = (f_l[:].unsqueeze(2)
                            .broadcast_to([B_LOC, OUT_NODES, OUT_DIM]))
                    nc.vector.tensor_tensor(
                        v_l[:].rearrange("p (j d) -> p j d", d=OUT_DIM),
                        sl[:].rearrange("p (j d) -> p j d", d=OUT_DIM),
                        f_lb, op=ALU.mult,
                    )
                    nc.sync.dma_start(out_d[:], v_l[:])

    nc.compile()
    return nc


def make_inmaps(x, W):
    x = np.ascontiguousarray(np.asarray(x, dtype=np.float32))
    W = np.ascontiguousarray(np.asarray(W, dtype=np.float32))
    # 16 8x8 blocks of 1/B on the diagonal
    ones_blk = (np.kron(np.eye(128 // IN_DIM, dtype=np.float32),
                        np.ones((IN_DIM, IN_DIM), dtype=np.float32)) / B)
    in_maps = []
    for cid in range(N_CORES):
        sh = slice(cid * I_LOC, (cid + 1) * I_LOC)
        x_sh = x[:, sh, :].reshape(B, IK)
        # [2, NT, 128, 128]: batch-half-major so iteration 0's half-0
        # matmuls are gated by only half the xT bytes
        xT = np.ascontiguousarray(
            x_sh.T.astype(ml_dtypes.bfloat16)
            .reshape(NT, 128, 2, 128).transpose(2, 0, 1, 3))
        xb = x_sh.astype(ml_dtypes.bfloat16).reshape(2, 128, IK)
        wb = W[sh].transpose(0, 3, 1, 2).reshape(NT, 128, JD).astype(
            ml_dtypes.bfloat16)
        in_maps.append({
            "xT": xT, "xb": xb, "wb": wb,
            "onesb": ones_blk.astype(ml_dtypes.bfloat16),
        })
    return in_maps


def assemble_output(per_core_outs):
    slices = [per_core_outs[c]["out"].reshape(B_LOC, OUT_NODES, OUT_DIM)
              for c in range(N_CORES)]
    v = np.concatenate(slices, axis=0)        # (256, 10, 16), already [b, j, d]
    return v[..., None].astype(np.float32)    # (256, 10, 16, 1)


_CACHED_NC = None


def kernel(x=None, W=None, **kw):
    global _CACHED_NC
    if x is None:
        x = kw["x"]
    if W is None:
        W = kw["W"]
    if _CACHED_NC is None:
        _CACHED_NC = build_nc()
    in_maps = make_inmaps(x, W)
    res = run_bass_kernel_spmd(
        _CACHED_NC, in_maps, core_ids=list(range(N_CORES)))
    return assemble_output(res.results)


if __name__ == "__main__":
    nc = build_nc()
    print("build + compile OK")
